# revision 36
# baseline (speedup 1.0000x reference)
"""Multi-head attention (B=2, S=2048, E=1024, H=16, D=64) on 8 trn2 cores.

Sharding: core c = (b, g) with b = c // 4 (batch), g = c % 4 (head group of
4 heads = 256 features). Each core computes Q/K/V projections for its head
group, full attention for its 4 heads, and a partial output projection
(columns of its group); a ReduceScatter over the 4 cores of each batch sums
the partials and leaves each core with a [512, 1024] slice of the final
output. The host concatenates the slices and adds bo.

Device-side layouts (host pre-transposes/casts):
  xT  [1024, 2048]  query[b].T                 (compute dtype)
  wqT/wkT/wvT [1024, 256]  W[g*256:(g+1)*256, :].T
  woT [256, 1024]          Wo[:, g*256:(g+1)*256].T
  bkq_c [128, 4]           K/Q bias columns (fp32, added in PSUM->SBUF copy)
  bv_b [128, 256]          V bias row pre-broadcast over partitions

On-chip dataflow per core (all contractions on the partition dim):
  Q^T,K^T [f,s] = (W^T chunk).T @ x^T + bias   (bias via DVE tensor_scalar)
  V [k,f] = (x^T chunk).T @ W^T + bias         (bias via DVE tensor_tensor)
  S^T [k,q] = (K^T chunk).T @ Q^T   (K = d = 64)
  P^T = exp(S^T / 8)  via ScalarE, PSUM -> SBUF, cast to compute dtype
  O  [q,d+1] = (P^T chunk).T @ V'   with V' = [V | 1] (col d = denom)
    -- flipped AV: stationary = P^T chunk, so the matmul's free dim is
       d+1 = 65 instead of 512, quartering tensor-engine time there.
  O <- O * (1/denom)  (DVE per-partition scalar multiply), then
  O^T via DMA crossbar transpose back into SBUF (idle DMA engines)
  Y [s,f] = (O^T chunk).T @ Wo^T  -> ReduceScatter(+) over the 4-core group
  (Y is written bf16; the host accumulates/adds bo in fp32.)

Scheduling: engine queues are in-order, so emission order is everything.
Dummy matmuls warm the PE p-state during the input-DMA wait; x arrives in
column groups and the K projection is split into half-tiles emitted just
ahead of the score group that first needs each one, so ScalarE starts
~10us in and then runs nearly stall-free; every iteration interleaves the
previous q-chunk's AV (split per 128-query subblock) between score groups
with a half-block skew that keeps each exp window's tensor-engine load
under budget; the last q-chunk's AV rides inside the final iteration so
the tail owes only one head + transposes + out-projection.
"""

import numpy as np

B, S, E, H, D = 2, 2048, 1024, 16, 64
G = 4            # head groups (tensor-parallel)
GH = H // G      # heads per group = 4
GF = GH * D      # features per group = 256
NC = 8
SCALE = 1.0 / np.sqrt(D)
# fp8 projections: x and the QKV weights are sent as 2-term fp8 hi/lo splits
# (3 DoubleRow products = 0.75x the bf16 tensor-engine cost, ~0.6x the
# error).  Power-of-2 prescales keep the lo terms out of the fp8 subnormal
# range; the exp scale and a host-side Wo/(SX*SW) make them cancel exactly.
SX = 8.0         # x prescale
SW = 16.0        # Wq/Wk/Wv prescale
SCALE_EXP = SCALE / (SX * SW) ** 2

_CACHE = {}


def _build(mode: str, collective: bool, reps: int = 1):
    import concourse.bass as bass
    import concourse.mybir as mybir
    import concourse.tile as tile
    from concourse import bacc

    dt = mybir.dt
    C = {"bf16": dt.bfloat16, "f32r": dt.float32r, "fp32": dt.float32}[mode]
    f32 = dt.float32
    F8 = dt.float8e4
    DR = mybir.MatmulPerfMode.DoubleRow

    nc = bacc.Bacc()

    # hi/lo pairs are packed in one dram tensor (hi | lo along the row) and
    # weights arrive host-preshuffled to the SBUF layout [128, EC*GF], so
    # each load is one contiguous full-bandwidth DMA and the issue count
    # (625ns of serialized HWDGE each) stays low.
    EC_ = E // 128
    xT8 = nc.dram_tensor("xT8", [128, 2 * (E // 128) * S], F8,
                     kind="ExternalInput")
    wqT8 = nc.dram_tensor("wqT8", [128, 2 * EC_ * GF], F8,
                          kind="ExternalInput")
    wkT8 = nc.dram_tensor("wkT8", [128, 2 * EC_ * GF], F8,
                          kind="ExternalInput")
    wvT8 = nc.dram_tensor("wvT8", [128, 2 * EC_ * GF], F8,
                          kind="ExternalInput")
    woT = nc.dram_tensor("woT", [128, 2 * E], C, kind="ExternalInput")
    bkq_c = nc.dram_tensor("bkq_c", [128, 4], f32, kind="ExternalInput")
    bv_b = nc.dram_tensor("bv_b", [128, GF], C, kind="ExternalInput")
    # bf16 output halves the output DMA time; the host accumulates in fp32
    if collective:
        yout = nc.dram_tensor("yout", [S // G, E], C, kind="ExternalOutput")
    else:
        yout = nc.dram_tensor("yout", [S, E], C, kind="ExternalOutput")

    EC = E // 128    # 8 e-chunks
    QC = S // 512    # 4 q-chunks
    KB = S // 128    # 16 k-blocks
    VW = GH * (D + 1)  # 260: V' row width (per head: 64 data + 1 ones col)

    def emit_body(nc, tc, res, do_coll):
        # x as one [hi | lo] fp8 resident tile, (e-chunk, hi/lo)-major;
        # column-group DMAs fill hi and lo together
        xAll = res.tile([128, 2 * EC * S], F8, tag="xAll", name="xAll")
        x4 = xAll[:].rearrange("p (g t s) -> p g t s", t=2, s=S)
        xhv = x4[:, :, 0, :]
        xlv = x4[:, :, 1, :]

        # Q/K weights are (fb, t)-blocked so each fb half is one contiguous
        # DMA and the first projection only waits on its own fb block
        wq8_sb = res.tile([128, 2 * EC * GF], F8, tag="wq8")
        wk8_sb = res.tile([128, 2 * EC * GF], F8, tag="wk8")
        wv8_sb = res.tile([128, 2 * EC * GF], F8, tag="wv8")
        wq5 = wq8_sb[:].rearrange("p (fb t g f) -> p fb t g f",
                                  fb=2, t=2, g=EC)
        wk5 = wk8_sb[:].rearrange("p (fb t g f) -> p fb t g f",
                                  fb=2, t=2, g=EC)
        wv4 = wv8_sb[:].rearrange("p (t g f) -> p t g f", t=2, g=EC)
        wv_pair = (wv4[:, 0], wv4[:, 1])
        woT_sb = res.tile([128, 2 * E], C, tag="woT")
        QT_sb = [[res.tile([128, 512], C, tag=f"QT{fb}_{qc}", name=f"QT{fb}_{qc}")
                  for qc in range(QC)] for fb in range(2)]
        KT_sb = [[res.tile([128, 512], C, tag=f"KT{fb}_{qc}", name=f"KT{fb}_{qc}")
                  for qc in range(QC)] for fb in range(2)]
        V_sb = [res.tile([128, VW], C, tag=f"V{kb}", name=f"V{kb}")
                for kb in range(KB)]
        # O^T per qc: [128, 2*512]: free = hb*512 + q  (hb = head-pair block)
        OT2_sb = [res.tile([128, 2 * 512], C, tag=f"OT{qc}", name=f"OT{qc}")
                  for qc in range(QC)]
        bkq_sb = res.tile([128, 4], f32, tag="bkq")
        bk_sb = bkq_sb[:, 0:2]
        bq_sb = bkq_sb[:, 2:4]
        bv_sb = res.tile([128, GF], C, tag="bv")

        # The PE warmup's scrap tile must be ready before Pool's DMAs queue
        scrap = res.tile([1, 128], C, tag="scrap")
        nc.gpsimd.memset(scrap[:], 0.0)

        # input DMAs: each engine queue serializes issue+transfer, so the
        # load is split across SP (critical prefix + weights), DVE and Pool
        # (later x column groups), all concurrent in wall time.
        xsrc = xT8[:].rearrange("p (g t s) -> p g t s", t=2, s=S)

        def dma_xcols(eng, t, c0, c1):
            # one hi (t=0) or lo (t=1) plane of a column group
            eng.dma_start(out=x4[:, :, t, c0:c1], in_=xsrc[:, :, t, c0:c1])

        def dma_wfb(src, dst_sb, fb):
            nc.sync.dma_start(out=dst_sb[:, fb * 2048:(fb + 1) * 2048],
                              in_=src[:, fb * 2048:(fb + 1) * 2048])

        nc.sync.dma_start(out=bkq_sb[:], in_=bkq_c[:])
        dma_wfb(wkT8, wk8_sb, 0)
        dma_xcols(nc.sync, 0, 0, 512)
        dma_wfb(wqT8, wq8_sb, 0)
        dma_xcols(nc.sync, 1, 0, 512)
        dma_wfb(wkT8, wk8_sb, 1)
        dma_wfb(wqT8, wq8_sb, 1)
        nc.sync.dma_start(out=bv_sb[:], in_=bv_b[:])
        nc.sync.dma_start(out=wv8_sb[:], in_=wvT8[:])
        nc.sync.dma_start(out=woT_sb[:], in_=woT[:])
        dma_xcols(nc.scalar, 0, 512, 1024)
        dma_xcols(nc.scalar, 1, 512, 1024)
        for qp in range(2, 4):
            dma_xcols(nc.gpsimd, 0, qp * 512, qp * 512 + 512)
            dma_xcols(nc.gpsimd, 1, qp * 512, qp * 512 + 512)

        # ones columns of V'
        for kb in range(KB):
            nc.gpsimd.memset(
                V_sb[kb][:].rearrange("p (h x) -> p h x", x=D + 1)[:, :, D:D + 1],
                1.0)

        # ---- emit helpers ----
        def emit_qk_proj(pp, w5, dst, b_sb, fb, qc, c0=0, c1=512):
            # Q^T / K^T group in [f, s] layout: stationary = W^T chunk.
            # fp8 DoubleRow, 3 compensated products, e-chunks paired.
            w = c1 - c0
            pq = pp.tile([128, 512], f32, tag="pq", name="pq", bufs=2)
            wh, wl = w5[:, fb, 0], w5[:, fb, 1]
            prods = ((wh, xhv), (wl, xhv), (wh, xlv))
            for cc in range(w // 256):
                q0 = qc * 512 + c0 + cc * 256
                for pi, (wv, xv) in enumerate(prods):
                    for j in range(EC // 2):
                        nc.tensor.matmul(
                            pq[:, cc * 256:cc * 256 + 256],
                            lhsT=wv[:, 2 * j:2 * j + 2, :],
                            rhs=xv[:, 2 * j:2 * j + 2, q0:q0 + 256],
                            perf_mode=DR,
                            start=(pi == 0 and j == 0),
                            stop=(pi == 2 and j == EC // 2 - 1))
            # bias folded into the PSUM->SBUF copy as a per-partition scalar
            nc.vector.tensor_scalar_add(
                out=dst[fb][qc][:, c0:c1], in0=pq[:, 0:w],
                scalar1=b_sb[:, fb:fb + 1])

        def emit_v_proj_pair(pp, j):
            # V group in natural [k, f] layout for k-blocks 2j, 2j+1
            pv = pp.tile([128, 512], f32, tag="pav", name="pv", bufs=2)
            wvh, wvl = wv_pair
            prods = ((xhv, wvh), (xhv, wvl), (xlv, wvh))
            for t in range(2):
                kb = 2 * j + t
                for pi, (xv, wv) in enumerate(prods):
                    for jj in range(EC // 2):
                        nc.tensor.matmul(
                            pv[:, t * GF:(t + 1) * GF],
                            lhsT=xv[:, 2 * jj:2 * jj + 2,
                                    kb * 128:kb * 128 + 128],
                            rhs=wv[:, 2 * jj:2 * jj + 2, :],
                            perf_mode=DR,
                            start=(pi == 0 and jj == 0),
                            stop=(pi == 2 and jj == EC // 2 - 1))
            for t in range(2):
                kb = 2 * j + t
                # bias via host-broadcast row, fused into the PSUM->SBUF copy
                nc.vector.tensor_tensor(
                    out=V_sb[kb][:].rearrange(
                        "p (h x) -> p h x", x=D + 1)[:, :, 0:D],
                    in0=pv[:, t * GF:(t + 1) * GF].rearrange(
                        "p (h d) -> p h d", d=D),
                    in1=bv_sb[:].rearrange("p (h d) -> p h d", d=D),
                    op=mybir.AluOpType.add)

        def emit_score_group(ps, h, qc, ptt, gi):
            hb, hr = h // 2, (h % 2) * D
            kb0 = 2 * gi
            pst = ps.tile([128, 2 * 512], f32, tag="pst", name="pst", bufs=2)
            for kj in range(2):
                kb = kb0 + kj
                nc.tensor.matmul(
                    pst[:, kj * 512:(kj + 1) * 512],
                    lhsT=KT_sb[hb][kb // 4][hr:hr + D,
                                            (kb % 4) * 128:(kb % 4) * 128 + 128],
                    rhs=QT_sb[hb][qc][hr:hr + D, :],
                    start=True, stop=True)
            nc.scalar.activation(
                ptt[:, kb0 * 512:(kb0 + 2) * 512],
                pst[:],
                mybir.ActivationFunctionType.Exp, scale=SCALE_EXP)

        def emit_av_qs(pav, recp, O2, h, ptt, qs, kb0=0, kb1=None):
            # Flipped AV for one 128-query subblock: stationary = P^T chunk
            # [128k, 128q], moving = V' [128k, 65]; accumulate over k-blocks.
            kb1 = KB if kb1 is None else kb1
            for kb in range(kb0, kb1):
                nc.tensor.matmul(
                    pav[:, qs * (D + 1):(qs + 1) * (D + 1)],
                    lhsT=ptt[:, kb * 512 + qs * 128:kb * 512 + qs * 128 + 128],
                    rhs=V_sb[kb][:, h * (D + 1):(h + 1) * (D + 1)],
                    start=(kb == 0), stop=(kb == KB - 1))
            if kb1 < KB:
                return
            rec = recp.tile([128, 1], f32, tag="rec", name="rec")
            nc.vector.reciprocal(
                rec[:], pav[:, qs * (D + 1) + D:qs * (D + 1) + D + 1])
            nc.vector.tensor_scalar_mul(
                out=O2[qs][:, h * D:(h + 1) * D],
                in0=pav[:, qs * (D + 1):qs * (D + 1) + D],
                scalar1=rec[:])

        def emit_transposes(pp, O2, qc, hb, eng=None):
            # O [q, f-pair] -> O^T [f-pair, q] for head pair hb, all 4 qs,
            # via the DMA crossbar transpose (idle DMA engines, no PE/DVE)
            eng = eng or nc.sync
            for qs in range(4):
                eng.dma_start_transpose(
                    out=OT2_sb[qc][:, hb * 512 + qs * 128:
                                   hb * 512 + qs * 128 + 128],
                    in_=O2[qs][:, hb * 128:hb * 128 + 128])

        def emit_outproj_sb(po, ysb, sb, act_copy=False, tags=("pav", "pq")):
            qc = sb // 4
            for fc in range(2):
                pyt = po.tile([128, 512], f32, tag=tags[fc], name="pyt")
                for ec in range(2):
                    nc.tensor.matmul(
                        pyt[:],
                        lhsT=OT2_sb[qc][:, ec * 512 + (sb % 4) * 128:
                                        ec * 512 + (sb % 4) * 128 + 128],
                        rhs=woT_sb[:, ec * E + fc * 512:ec * E + fc * 512 + 512],
                        start=(ec == 0), stop=(ec == 1))
                yt = ysb.tile([128, 512], C, tag="yt", name="yt")
                if act_copy and fc == 1:
                    # tail only: ScalarE is idle once the exps are done
                    nc.scalar.copy(yt[:], pyt[:])
                else:
                    nc.vector.tensor_copy(yt[:], pyt[:])
                dst = y_part if collective else yout
                nc.sync.dma_start(
                    out=dst[sb * 128:(sb + 1) * 128, fc * 512:(fc + 1) * 512],
                    in_=yt[:])

        # ---- emission (order = scheduler priority; engine queues are
        # in-order, so every dependency must appear before its consumer,
        # and slow-to-unblock work must not be emitted ahead of urgent
        # work on the same engine) ----
        # PSUM (8 banks): "pst" [128,1024] x2 = 4 banks (scores+exp),
        # "pq" [128,512] x2 = 2 banks (projections, pT staging, outproj),
        # "pav" [128,512] x2 = 2 banks (V-proj, AV accum, outproj).
        with tc.tile_pool(name="dram", bufs=1, space="DRAM") as dram, \
             tc.tile_pool(name="pall", bufs=2, space="PSUM") as pall, \
             tc.tile_pool(name="ptp", bufs=6) as ptp, \
             tc.tile_pool(name="o2p", bufs=3) as o2p, \
             tc.tile_pool(name="recp", bufs=8) as recp, \
             tc.tile_pool(name="ysb", bufs=6) as ysb:
            if collective:
                y_part = dram.tile([S, E], C, tag="ypart")
                rs_out = dram.tile([S // G, E], C, tag="rsout")

            def new_ptt(h):
                return ptp.tile([128, KB * 512], C, tag="ptt", name=f"ptt{h}")

            def exp_block(h, qc, ptt, extras):
                """Emit the 8 score groups + exp calls for (h, qc), with
                `extras` (list of thunks) interleaved between groups."""
                for gi in range(8):
                    emit_score_group(pall, h, qc, ptt, gi)
                    if gi >= 1 and extras:
                        extras.pop(0)()
                while extras:
                    extras.pop(0)()

            # --- q-chunk 0 phase: K/Q projections interleaved with the
            # first heads' score groups so ScalarE starts ASAP.
            def kp(fb, kc, half):
                return lambda: emit_qk_proj(pall, wk5, KT_sb, bk_sb, fb,
                                            kc, half * 256, half * 256 + 256)

            ptts = {}
            # warm the tensor engine's p-state during the input-DMA wait:
            # back-to-back trivial matmuls keep it busy so the first real
            # projections run at full clock
            pwu = pall.tile([128, 2 * 512], f32, tag="pst", name="pwu", bufs=2)
            for _ in range(112):
                nc.tensor.matmul(pwu[0:1, 0:128], lhsT=scrap[:, 0:1],
                                 rhs=scrap[:], start=True, stop=True)
            nc.vector.tensor_copy(scrap[:], pwu[0:1, 0:128])
            # K projection in half-tiles, each emitted just ahead of the
            # score group that first needs it (in-order PE queue = JIT feed)
            kp(0, 0, 0)()
            emit_qk_proj(pall, wq5, QT_sb, bq_sb, 0, 0)
            kp(0, 0, 1)()
            ptts[0] = new_ptt(0)
            exp_block(0, 0, ptts[0], [
                kp(0, 1, 0), kp(0, 1, 1), kp(0, 2, 0),
                kp(0, 2, 1), kp(0, 3, 0), kp(0, 3, 1)])
            ptts[1] = new_ptt(1)
            exp_block(1, 0, ptts[1], [
                lambda: emit_qk_proj(pall, wq5, QT_sb, bq_sb, 1, 0),
                kp(1, 0, 0), kp(1, 0, 1), kp(1, 1, 0), kp(1, 1, 1)])
            ptts[2] = new_ptt(2)
            exp_block(2, 0, ptts[2], [
                kp(1, 2, 0), kp(1, 2, 1), kp(1, 3, 0), kp(1, 3, 1),
                lambda: emit_v_proj_pair(pall, 0)])
            ptts[3] = new_ptt(3)
            exp_block(3, 0, ptts[3], [
                lambda: emit_v_proj_pair(pall, 1),
                lambda: emit_qk_proj(pall, wq5, QT_sb, bq_sb, 0, 1)])
            emit_qk_proj(pall, wq5, QT_sb, bq_sb, 1, 1)

            # steady-state: exps of qc overlap AV of qc-1 (split per qs),
            # transposes of qc-1 complete within the iteration, outproj of
            # qc-2 rides along; the tail only owes qc3's AV + outproj.
            def av_extras(O2, h, ptt):
                # pav is allocated lazily at the first AV chunk so no other
                # same-tag allocation can slip between tile() and first write
                holder = {}

                def mk(qs):
                    def go():
                        if "pav" not in holder:
                            holder["pav"] = pall.tile(
                                [128, 512], f32, tag="pav", name="pav", bufs=2)
                        emit_av_qs(holder["pav"], recp, O2, h, ptt, qs)
                    return go
                return [mk(qs) for qs in range(4)]

            # Global half-block skew: in iteration qc, block h0 carries
            # AV(qc-2, h3) and blocks h1..h3 carry AV(qc-1, h0..h2), so
            # per-block tensor-engine load stays under the exp window.
            O2s = {}
            ptth = {0: ptts}
            for qc in range(1, QC):
                pqc = qc - 1
                O2s[pqc] = [o2p.tile([128, GH * D], C, tag=f"o2_{qs}",
                                     name="o2") for qs in range(4)]
                new_ptts = {}
                new_ptts[0] = new_ptt(0)
                if qc == 1:
                    ex = [lambda: emit_v_proj_pair(pall, 2),
                          lambda: emit_v_proj_pair(pall, 3),
                          lambda: emit_v_proj_pair(pall, 4)]
                else:
                    ex = av_extras(O2s[qc - 2], 3, ptth[qc - 2][3])
                exp_block(0, qc, new_ptts[0], ex)
                if qc >= 2:
                    emit_transposes(pall, O2s[qc - 2], qc - 2, 1)
                new_ptts[1] = new_ptt(1)
                ex = av_extras(O2s[pqc], 0, ptth[pqc][0])
                if qc == 1:
                    # defer AV(qc0,h0) to a block-h2 prefix; h1 carries only V
                    av_h0_deferred = ex
                    ex = [lambda: emit_v_proj_pair(pall, 5),
                          lambda: emit_v_proj_pair(pall, 6),
                          lambda: emit_v_proj_pair(pall, 7)]
                if qc == QC - 1:
                    # last iteration: pull the whole backlog forward so the
                    # tail only owes AV(q3,h3) + transposes + outproj
                    ex = ex + av_extras(O2s[pqc], 3, ptth[pqc][3])
                exp_block(1, qc, new_ptts[1], ex)
                if qc >= 2:
                    for sb in range((qc - 2) * 4, (qc - 2) * 4 + 2):
                        emit_outproj_sb(pall, ysb, sb)
                new_ptts[2] = new_ptt(2)
                if qc == 1:
                    for thunk in av_h0_deferred:
                        thunk()
                ex = av_extras(O2s[pqc], 1, ptth[pqc][1])
                if qc < QC - 1:
                    ex.append(lambda: emit_qk_proj(
                        pall, wq5, QT_sb, bq_sb, 0, qc + 1))
                else:
                    O2s[qc] = [o2p.tile([128, GH * D], C, tag=f"o2_{qs}",
                                        name="o2") for qs in range(4)]
                    ex = ex + av_extras(O2s[pqc], 2, ptth[pqc][2])
                exp_block(2, qc, new_ptts[2], ex)
                if qc >= 2:
                    for sb in range((qc - 2) * 4 + 2, (qc - 2) * 4 + 4):
                        emit_outproj_sb(pall, ysb, sb)
                if qc == QC - 1:
                    emit_transposes(pall, O2s[pqc], pqc, 1)
                new_ptts[3] = new_ptt(3)
                if qc < QC - 1:
                    ex = av_extras(O2s[pqc], 2, ptth[pqc][2])
                    ex.append(lambda: emit_qk_proj(
                        pall, wq5, QT_sb, bq_sb, 1, qc + 1))
                else:
                    ex = (av_extras(O2s[qc], 0, new_ptts[0])
                          + av_extras(O2s[qc], 1, new_ptts[1])
                          + av_extras(O2s[qc], 2, new_ptts[2]))
                exp_block(3, qc, new_ptts[3], ex)
                emit_transposes(pall, O2s[pqc], pqc, 0)
                ptth[qc] = new_ptts
                ptts = new_ptts

            # tail: AV(q3,h3) is kb-split so only its last two matmuls
            # wait on the final exp; outproj(q2) overlaps the b3 exp drain;
            # outproj(q3) runs out of the freed pst banks.  The AV(3,3)
            # accumulator borrows the "pq" bank so outproj(q2) can rotate
            # through "pav" without aliasing a live accumulation.
            q3, q2 = QC - 1, QC - 2
            emit_transposes(pall, O2s[q3], q3, 0)
            holder33 = {}

            def av33(qs, k0, k1):
                if "pav" not in holder33:
                    holder33["pav"] = pall.tile([128, 512], f32, tag="pq",
                                                name="pav33", bufs=2)
                emit_av_qs(holder33["pav"], recp, O2s[q3], 3, ptth[q3][3],
                           qs, k0, k1)

            for qs in range(4):
                av33(qs, 0, 14)
            for sb in range(q2 * 4, q2 * 4 + 4):
                emit_outproj_sb(pall, ysb, sb, tags=("pav", "pav"))
            for qs in range(4):
                av33(qs, 14, KB)
            emit_transposes(pall, O2s[q3], q3, 1)
            for sb in range(q3 * 4, q3 * 4 + 4):
                emit_outproj_sb(pall, ysb, sb, act_copy=True,
                                tags=("pst", "pst"))

            if collective and do_coll:
                nc.gpsimd.collective_compute(
                    "ReduceScatter",
                    mybir.AluOpType.add,
                    replica_groups=[[0, 1, 2, 3], [4, 5, 6, 7]],
                    ins=[y_part.opt()],
                    outs=[rs_out.opt()],
                )
                nc.sync.dma_start(out=yout[:], in_=rs_out[:])

    with tile.TileContext(nc) as tc:
        with tc.tile_pool(name="res", bufs=1) as res:
            for _rep in range(reps):
                emit_body(nc, tc, res, do_coll=(_rep == reps - 1))
    nc.finalize()
    return nc


def _np_dtype(mode):
    if mode == "bf16":
        import ml_dtypes
        return ml_dtypes.bfloat16
    return np.float32


def _split8(a):
    # 2-term fp8 hi/lo split: hi = fp8(a), lo = fp8(a - hi)
    import ml_dtypes
    f8 = ml_dtypes.float8_e4m3
    hi = np.ascontiguousarray(a, np.float32).astype(f8)
    lo = (a - hi.astype(np.float32)).astype(f8)
    return hi, lo


def _shuf_w(a, nech):
    # [E', F] -> the SBUF-resident layout [128, (E'//128)*F]
    a = np.ascontiguousarray(a)
    e, f = a.shape
    assert e == nech * 128
    return np.ascontiguousarray(
        a.reshape(nech, 128, f).transpose(1, 0, 2).reshape(128, nech * f))


def _shuf_wfb(hi, lo):
    # hi/lo [E, 256] -> [128, (fb, t, g, f128)] with fb = feature half
    out = np.stack([
        hi.reshape(8, 128, 2, 128).transpose(1, 2, 0, 3),   # [p, fb, g, f]
        lo.reshape(8, 128, 2, 128).transpose(1, 2, 0, 3),
    ], axis=2)                                              # [p, fb, t, g, f]
    return np.ascontiguousarray(out.reshape(128, 4096))


def _in_maps(query, Wq, bq, Wk, bk, Wv, bv, Wo, bo, mode):
    ndt = _np_dtype(mode)
    SB = SX * SW  # Q/K/V (and their biases) come out scaled by this
    maps = []
    for c in range(NC):
        b, g = c // G, c % G
        gr = slice(g * GF, (g + 1) * GF)
        xh, xl = _split8(np.asarray(query[b], np.float32).T * SX)
        wqh, wql = _split8(np.asarray(Wq[gr, :], np.float32).T * SW)
        wkh, wkl = _split8(np.asarray(Wk[gr, :], np.float32).T * SW)
        wvh, wvl = _split8(np.asarray(Wv[gr, :], np.float32).T * SW)
        maps.append({
            "xT8": np.ascontiguousarray(np.stack(
                [xh.reshape(8, 128, S), xl.reshape(8, 128, S)],
                axis=2).transpose(1, 0, 2, 3).reshape(128, 2 * 8 * S)),
            "wqT8": _shuf_wfb(wqh, wql),
            "wkT8": _shuf_wfb(wkh, wkl),
            "wvT8": np.concatenate([_shuf_w(wvh, 8), _shuf_w(wvl, 8)], 1),
            "woT": _shuf_w(np.asarray(Wo[:, gr], np.float32).T / SB,
                           2).astype(ndt),
            "bkq_c": np.ascontiguousarray(np.concatenate([
                np.asarray(bk[gr], np.float32).reshape(2, 128).T,
                np.asarray(bq[gr], np.float32).reshape(2, 128).T],
                axis=1)) * SB,
            "bv_b": np.ascontiguousarray(
                np.tile(np.asarray(bv[gr], np.float32).reshape(1, GF) * SB,
                        (128, 1))).astype(ndt),
        })
    return maps


def kernel(query, Wq, bq, Wk, bk, Wv, bv, Wo, bo,
           mode="bf16", collective=True, trace=False):
    from concourse.bass_utils import run_bass_kernel_spmd

    key = (mode, collective, 1)
    if key not in _CACHE:
        _CACHE[key] = _build(mode, collective)
    nc = _CACHE[key]

    maps = _in_maps(query, Wq, bq, Wk, bk, Wv, bv, Wo, bo, mode)
    res = run_bass_kernel_spmd(nc, maps, list(range(NC)), trace=trace)

    out = np.empty((B, S, E), np.float32)
    if collective:
        for c in range(NC):
            b, g = c // G, c % G
            out[b, g * (S // G):(g + 1) * (S // G), :] = np.asarray(
                res.results[c]["yout"], np.float32)
    else:
        for b in range(B):
            out[b] = sum(np.asarray(res.results[b * G + g]["yout"], np.float32)
                         for g in range(G))
    out += np.asarray(bo, np.float32)
    if trace:
        kernel.last_results = res
    return out

